# revision 10
# baseline (speedup 1.0000x reference)
"""MLA decode kernel for 8 TRN2 NeuronCores (v5).

Sharding: batch-parallel - core b handles batch element b (B=8, n_cores=8).
All weights replicated per core, host-pretiled into partition-major
contiguous layouts so every DMA moves 4-18KB contiguous lines per
partition (the baseline's 1-2KB strided lines held HBM at ~33% MBU).

Key changes vs baseline:
- kv cache fed ONLY in [c,t] layout (kvT); the [t,c] layout for the value
  matmul is built on-chip with the DMA XBAR transpose (InstDmaTransposeAnt,
  14ns per 16x128 tile) - saves 8.4MB of HBM traffic per core.
- exp(scores) transposed for the value matmul via XBAR too, replacing
  64 PE transposes + 64 scalar copies.
- q_norm folded into wq_b rows on host; absorption restructured to
  16 weight-streaming matmuls + one XBAR instead of 64 stationary-load
  matmuls.
"""
import numpy as np
import ml_dtypes

import concourse.bacc as bacc
import concourse.mybir as mybir
from concourse import bass_utils
from concourse.tile import TileContext
from concourse.masks import make_identity

BF = mybir.dt.bfloat16
F32 = mybir.dt.float32
npbf = ml_dtypes.bfloat16

N_CORES = 8
B, S, DIM = 8, 1, 2048
H = 16
QLR, KVLR = 1536, 512
DN, DR, DV = 128, 64, 128
TP = 8191
T = TP + 1           # 8192 padded; col 8191 = injected new token
SCALE = float((DN + DR) ** -0.5)
EPS = 1e-6
TBW = 1024           # t-block width for kvT streaming
NTB = T // TBW       # 4 blocks
NSB = T // 512       # 16 score sub-blocks

_NC_CACHE = {}


def _build():
    if "nc" in _NC_CACHE:
        return _NC_CACHE["nc"]
    nc = bacc.Bacc("TRN2", target_bir_lowering=False, debug=False,
                   num_devices=N_CORES)
    I = {}

    def inp(name, shape, dt=BF):
        I[name] = nc.dram_tensor(name, shape, dt, kind="ExternalInput")
        return I[name]

    inp("xT16", [128, 16])
    inp("kvT4", [4, 128, T])
    inp("peT", [64, T])
    inp("wqa_p", [128, 3, 16, 512])
    inp("wqb_p", [128, 6, 12, 512])
    inp("wkva_p", [128, 2, 16, 288])
    inp("wbk_p", [128, 16, 512])
    inp("wbv_p", [128, 4, 2048])
    inp("woT_p", [128, 4, 16, 512])
    inp("kvnw", [1, KVLR], F32)
    inp("wqab", [1, QLR], F32)
    inp("wqbb", [1, H * (DN + DR)], F32)
    inp("wkvab", [1, KVLR + DR], F32)
    inp("wob", [1, DIM], F32)
    inp("cosq", [1, H * 32], F32)
    inp("sinq", [1, H * 32], F32)
    out_d = nc.dram_tensor("out", [1, DIM], F32, kind="ExternalOutput")

    with TileContext(nc) as tc:
        _program(nc, tc, I, out_d)
    nc.compile()
    _NC_CACHE["nc"] = nc
    return nc


def _program(nc, tc, I, out_d):
    AL = mybir.AluOpType
    AF = mybir.ActivationFunctionType

    with (
        tc.tile_pool(name="consts", bufs=1) as cp,
        tc.tile_pool(name="wstream", bufs=2) as wp,
        tc.tile_pool(name="wconst", bufs=1) as wc,
        tc.tile_pool(name="kvTp", bufs=2) as kvTp,
        tc.tile_pool(name="kvnp", bufs=2) as kvnp,
        tc.tile_pool(name="attn", bufs=3) as atp,
        tc.tile_pool(name="ps_scores", bufs=2, space="PSUM") as pps,
        tc.tile_pool(name="ps_acc", bufs=1, space="PSUM") as ppa,
        tc.tile_pool(name="ps_tr", bufs=2, space="PSUM") as ppt,
        tc.tile_pool(name="ps_stage", bufs=2, space="PSUM") as ppg,
    ):
        id_f = cp.tile([128, 128], F32)
        make_identity(nc, id_f[:])

        def load_const(name, dt=F32):
            t = cp.tile(list(I[name].shape), dt, tag=name)
            nc.sync.dma_start(out=t[:], in_=I[name].ap())
            return t

        xT = load_const("xT16", BF)
        kvnw = load_const("kvnw")
        wqab = load_const("wqab")
        wqbb = load_const("wqbb")
        wkvab = load_const("wkvab")
        wob = load_const("wob")
        cosq = load_const("cosq")
        sinq = load_const("sinq")

        # big resident tiles
        peT_sb = wc.tile([64, T], BF)
        nc.sync.dma_start(out=peT_sb[:], in_=I["peT"].ap())
        wbk_sb = wc.tile([128, 16, 512], BF)
        nc.sync.dma_start(out=wbk_sb[:], in_=I["wbk_p"].ap())
        # ---- GEMV: y[1,M] f32 = x^T @ W (+bias); W host-pretiled ----
        def gemv_pret(w_name, nblocks, nk, mwtot, bias_sb, out_sb, x_sb):
            wd = I[w_name].ap()
            off = 0
            for mb in range(nblocks):
                wt = wp.tile([128, 16, 512], BF, tag="wstream")
                nc.sync.dma_start(out=wt[:, :nk, :mwtot], in_=wd[:, mb])
                for mb0 in range(0, mwtot, 512):
                    mw = min(512, mwtot - mb0)
                    ps = ppg.tile([1, 512], F32, tag="stage")
                    for kc in range(nk):
                        nc.tensor.matmul(
                            ps[:, :mw], x_sb[:, kc:kc + 1],
                            wt[:, kc, mb0:mb0 + mw],
                            start=(kc == 0), stop=(kc == nk - 1),
                        )
                    nc.vector.tensor_tensor(
                        out=out_sb[:, off + mb0:off + mb0 + mw], in0=ps[:, :mw],
                        in1=bias_sb[:, off + mb0:off + mb0 + mw], op=AL.add,
                    )
                off += mwtot

        def rmsnorm(in_view, N, w_sb, out_sb, tag):
            sq = cp.tile([1, 1536], F32, tag="scratch")
            ssq = cp.tile([1, 1], F32, tag=f"ssq{tag}")
            nc.scalar.activation(out=sq[:, :N], in_=in_view, func=AF.Square,
                                 accum_out=ssq[:])
            ms = cp.tile([1, 1], F32, tag=f"ms{tag}")
            nc.vector.tensor_scalar(out=ms[:], in0=ssq[:], scalar1=1.0 / N,
                                    scalar2=EPS, op0=AL.mult, op1=AL.add)
            sd = cp.tile([1, 1], F32, tag=f"sd{tag}")
            nc.scalar.activation(out=sd[:], in_=ms[:], func=AF.Sqrt)
            rstd = cp.tile([1, 1], F32, tag=f"rstd{tag}")
            nc.vector.reciprocal(out=rstd[:], in_=sd[:])
            if w_sb is not None:
                tmp = cp.tile([1, 1536], F32, tag="scratch")
                nc.vector.tensor_tensor(out=tmp[:, :N], in0=in_view,
                                        in1=w_sb[:, :N], op=AL.mult)
                src = tmp[:, :N]
            else:
                src = in_view
            nc.vector.tensor_scalar(out=out_sb, in0=src,
                                    scalar1=rstd[:], scalar2=None, op0=AL.mult)

        def trans_row(in_view, n, ps_out):
            nc.tensor.transpose(ps_out, in_view, id_f[0:1, 0:1])

        # ================= Q branch =================
        qa = cp.tile([1, QLR], F32)
        gemv_pret("wqa_p", 3, 16, 512, wqab, qa, xT)
        qan = cp.tile([1, QLR], F32)
        rmsnorm(qa[:], QLR, None, qan[:], "qa")

        pt_qa = ppt.tile([128, 64], F32, tag="tr")
        for kc in range(12):
            trans_row(qan[:, kc * 128:(kc + 1) * 128], 128,
                      pt_qa[:, kc:kc + 1])
        qaT = cp.tile([128, 12], BF)
        nc.scalar.copy(out=qaT[:], in_=pt_qa[:, :12])

        q = cp.tile([1, H * (DN + DR)], F32)
        gemv_pret("wqb_p", 6, 12, 512, wqbb, q, qaT)

        # rope(q_pe) across all heads
        qv = q[:].rearrange("b (h r) -> b h r", h=H)
        xr = qv[:, :, 128:192:2]
        xi = qv[:, :, 129:192:2]
        cosv = cosq[:].rearrange("b (h j) -> b h j", h=H)
        sinv = sinq[:].rearrange("b (h j) -> b h j", h=H)
        rp = cp.tile([1, H * DR], F32)
        rpv = rp[:].rearrange("b (h r) -> b h r", h=H)
        s1 = cp.tile([1, 512], F32, tag="rs1")
        s2 = cp.tile([1, 512], F32, tag="rs2")
        s1v = s1[:].rearrange("b (h j) -> b h j", h=H)
        s2v = s2[:].rearrange("b (h j) -> b h j", h=H)
        nc.vector.tensor_tensor(out=s1v, in0=xi, in1=sinv, op=AL.mult)
        nc.vector.tensor_tensor(out=s2v, in0=xr, in1=cosv, op=AL.mult)
        nc.vector.tensor_tensor(out=rpv[:, :, 0:64:2], in0=s2v, in1=s1v,
                                op=AL.subtract)
        nc.vector.tensor_tensor(out=s1v, in0=xr, in1=sinv, op=AL.mult)
        nc.vector.tensor_tensor(out=s2v, in0=xi, in1=cosv, op=AL.mult)
        nc.vector.tensor_tensor(out=rpv[:, :, 1:64:2], in0=s1v, in1=s2v,
                                op=AL.add)

        # q_nope columns -> qnT [128, 16] bf16
        pt_qn = ppt.tile([128, 64], F32, tag="tr")
        for h in range(H):
            trans_row(q[:, h * 192:h * 192 + 128], 128, pt_qn[:, h:h + 1])
        qnT = cp.tile([128, H], BF)
        nc.scalar.copy(out=qnT[:], in_=pt_qn[:, :H])

        # absorption: qT[cc][128c, 16h] = sum_d wbk[h,d,c] q_nope[h,d]
        qTx = cp.tile([128, 4, 16], BF)
        for cc in range(4):
            ps_ab = ppt.tile([128, 64], F32, tag="tr")
            for h in range(H):
                nc.tensor.matmul(
                    ps_ab[:, h:h + 1],
                    wbk_sb[:, h, cc * 128:(cc + 1) * 128],
                    qnT[:, h:h + 1], start=True, stop=True,
                )
            nc.scalar.copy(out=qTx[:, cc, :], in_=ps_ab[:, :H])

        # q_pe transposed -> qpT [64, 16]
        pt_qp = ppt.tile([128, 64], F32, tag="tr")
        for h in range(H):
            trans_row(rp[:, h * 64:(h + 1) * 64], 64, pt_qp[:64, h:h + 1])
        qpT = cp.tile([64, H], BF)
        nc.scalar.copy(out=qpT[:], in_=pt_qp[:64, :H])

        # ================= KV branch (new token) =================
        kvpe = cp.tile([1, KVLR + DR], F32)
        gemv_pret("wkva_p", 2, 16, 288, wkvab, kvpe, xT)
        kvn_f = cp.tile([1, KVLR], F32)
        rmsnorm(kvpe[:, :KVLR], KVLR, kvnw, kvn_f[:], "kv")

        kpe = cp.tile([1, DR], F32)
        kxr = kvpe[:, KVLR + 0:KVLR + 64:2]
        kxi = kvpe[:, KVLR + 1:KVLR + 64:2]
        ks1 = cp.tile([1, 32], F32, tag="krs1")
        ks2 = cp.tile([1, 32], F32, tag="krs2")
        nc.vector.tensor_tensor(out=ks1[:], in0=kxi, in1=sinq[:, :32], op=AL.mult)
        nc.vector.tensor_tensor(out=ks2[:], in0=kxr, in1=cosq[:, :32], op=AL.mult)
        nc.vector.tensor_tensor(out=kpe[:, 0:64:2], in0=ks2[:], in1=ks1[:],
                                op=AL.subtract)
        nc.vector.tensor_tensor(out=ks1[:], in0=kxr, in1=sinq[:, :32], op=AL.mult)
        nc.vector.tensor_tensor(out=ks2[:], in0=kxi, in1=cosq[:, :32], op=AL.mult)
        nc.vector.tensor_tensor(out=kpe[:, 1:64:2], in0=ks1[:], in1=ks2[:],
                                op=AL.add)

        kvn_bf = cp.tile([1, KVLR], BF)
        nc.scalar.copy(out=kvn_bf[:], in_=kvn_f[:])
        pt_kv = ppt.tile([128, 64], F32, tag="tr")
        for cc in range(4):
            trans_row(kvn_f[:, cc * 128:(cc + 1) * 128], 128,
                      pt_kv[:, cc:cc + 1])
        trans_row(kpe[:], 64, pt_kv[:64, 4:5])
        kvnT = cp.tile([128, 4], BF)
        nc.scalar.copy(out=kvnT[:], in_=pt_kv[:, :4])
        kpeT = cp.tile([64, 1], BF)
        nc.scalar.copy(out=kpeT[:], in_=pt_kv[:64, 4:5])

        # inject new token into pe cache (t=8191)
        nc.vector.tensor_copy(out=peT_sb[:, T - 1:T], in_=kpeT[:])

        # ================= attention =================
        den = cp.tile([H, NSB], F32)
        po = ppa.tile([H, 512], F32)
        kvT_d = I["kvT4"].ap()
        n_mm2 = 64
        mm2_i = 0
        for tb in range(NTB):
            t0 = tb * TBW
            kvTt = kvTp.tile([128, 4, TBW], BF, tag="kvT")
            nc.sync.dma_start(
                out=kvTt[:],
                in_=kvT_d[:, :, t0:t0 + TBW].rearrange("c p t -> p c t"),
            )
            # build kvn chunks for this block via XBAR (scalar queue)
            kvn_sb = kvnp.tile([128, TBW // 128, 4, 128], BF, tag="kvn")
            for cc in range(4):
                nc.scalar.dma_start(
                    out=kvn_sb[:, :, cc, :],
                    in_=kvTt[:, cc, :], transpose=True,
                )
            if tb == NTB - 1:
                # inject new token column (t=8191) into kvT and kvn
                for cc in range(4):
                    nc.vector.tensor_copy(out=kvTt[:, cc, TBW - 1:TBW],
                                          in_=kvnT[:, cc:cc + 1])
                nc.sync.dma_start(
                    out=kvn_sb[127:128, TBW // 128 - 1, :, :],
                    in_=kvn_bf[:].rearrange("a (c m) -> a c m", c=4),
                )

            for s in range(TBW // 512):
                sb_i = tb * (TBW // 512) + s
                ps = pps.tile([H, 512], F32, tag="scores")
                for cc in range(4):
                    nc.tensor.matmul(
                        ps[:], qTx[:, cc, :], kvTt[:, cc, s * 512:(s + 1) * 512],
                        start=(cc == 0), stop=False,
                    )
                nc.tensor.matmul(ps[:], qpT[:],
                                 peT_sb[:, t0 + s * 512:t0 + (s + 1) * 512],
                                 start=False, stop=True)
                ex = atp.tile([H, 512], BF, tag="exp")
                nc.scalar.activation(out=ex[:], in_=ps[:], func=AF.Exp,
                                     scale=SCALE,
                                     accum_out=den[:, sb_i:sb_i + 1])
                exT = atp.tile([128, 4, 16], BF, tag="expT")
                nc.scalar.dma_start(out=exT[:], in_=ex[:], transpose=True)
                for u in range(4):
                    nc.tensor.matmul(
                        po[:], exT[:, u, :], kvn_sb[:, s * 4 + u, :, :],
                        start=(mm2_i == 0), stop=(mm2_i == n_mm2 - 1),
                        skip_group_check=True,
                    )
                    mm2_i += 1

        den1 = cp.tile([H, 1], F32)
        nc.vector.tensor_reduce(out=den1[:], in_=den[:],
                                axis=mybir.AxisListType.X, op=AL.add)
        dinv = cp.tile([H, 1], F32)
        nc.vector.reciprocal(out=dinv[:], in_=den1[:])
        oln = cp.tile([H, 512], BF)
        nc.vector.tensor_scalar(out=oln[:], in0=po[:], scalar1=dinv[:],
                                scalar2=None, op0=AL.mult)

        # olT via XBAR: [16, 512] -> [128, 4, 16]
        olT = cp.tile([128, 4, 16], BF)
        nc.scalar.dma_start(out=olT[:], in_=oln[:], transpose=True)

        # V projection -> oT [128d, 16h] bf16
        wbv_sb = wc.tile([128, 4, 2048], BF)
        nc.sync.dma_start(out=wbv_sb[:], in_=I["wbv_p"].ap())
        ps_vo = ppt.tile([128, 64], F32, tag="tr")
        for h in range(H):
            for cc in range(4):
                nc.tensor.matmul(
                    ps_vo[:, h:h + 1],
                    wbv_sb[:, cc, h * 128:(h + 1) * 128],
                    olT[:, cc, h:h + 1],
                    start=(cc == 0), stop=(cc == 3),
                )
        oT = cp.tile([128, H], BF)
        nc.scalar.copy(out=oT[:], in_=ps_vo[:, :H])

        # wo projection -> out [1, 2048] f32
        woT_d = I["woT_p"].ap()
        for mb in range(4):
            wt = wp.tile([128, 16, 512], BF, tag="wstream")
            nc.sync.dma_start(out=wt[:], in_=woT_d[:, mb])
            ps = ppg.tile([1, 512], F32, tag="stage")
            for h in range(H):
                nc.tensor.matmul(ps[:], oT[:, h:h + 1], wt[:, h, :],
                                 start=(h == 0), stop=(h == H - 1))
            ob = cp.tile([1, 512], F32, tag="outb")
            nc.vector.tensor_tensor(
                out=ob[:], in0=ps[:],
                in1=wob[:, mb * 512:(mb + 1) * 512], op=AL.add,
            )
            nc.sync.dma_start(out=out_d.ap()[:, mb * 512:(mb + 1) * 512],
                              in_=ob[:])


def _prep_inputs(inputs):
    f = {k: np.asarray(v) for k, v in inputs.items()}
    x = f["x"].astype(np.float32).reshape(B, DIM)
    kvp = f["kv_cache_prefix"].astype(np.float32)
    pep_ = f["pe_cache_prefix"].astype(np.float32)
    cos = f["freqs_cos"].astype(np.float32).reshape(-1)[:32]
    sin = f["freqs_sin"].astype(np.float32).reshape(-1)[:32]

    wq_a = f["wq_a_w"].astype(np.float32)          # [1536, 2048]
    wq_b = f["wq_b_w"].astype(np.float32)          # [3072, 1536]
    qnw = f["q_norm_w"].astype(np.float32)
    wkv_a = f["wkv_a_w"].astype(np.float32)        # [576, 2048]
    wkv_b = f["wkv_b_w"].astype(np.float32).reshape(H, DN + DV, KVLR)
    wo = f["wo_w"].astype(np.float32)              # [2048, 2048]

    # pretiled layouts: [p, mb, k, m] with per-(p) contiguous (mb,k,m) lines
    wqa_p = np.ascontiguousarray(
        wq_a.reshape(3, 512, 16, 128).transpose(3, 0, 2, 1)).astype(npbf)
    wqbw = wq_b * qnw[None, :]                     # fold q_norm into wq_b
    wqb_p = np.ascontiguousarray(
        wqbw.reshape(6, 512, 12, 128).transpose(3, 0, 2, 1)).astype(npbf)
    wkva_p = np.ascontiguousarray(
        wkv_a.reshape(2, 288, 16, 128).transpose(3, 0, 2, 1)).astype(npbf)
    wbk_p = np.ascontiguousarray(
        wkv_b[:, :DN].transpose(1, 0, 2)).astype(npbf)   # [128d, 16h, 512c]
    wbv_p = np.ascontiguousarray(
        wkv_b[:, DN:].reshape(H, DV, 4, 128).transpose(3, 2, 0, 1)
        .reshape(128, 4, H * DV)).astype(npbf)           # [128c, 4cc, (h,d)]
    woT_p = np.ascontiguousarray(
        wo.T.reshape(16, 128, 4, 512).transpose(1, 2, 0, 3)).astype(npbf)

    shared = {
        "wqa_p": wqa_p, "wqb_p": wqb_p, "wkva_p": wkva_p,
        "wbk_p": wbk_p, "wbv_p": wbv_p, "woT_p": woT_p,
        "kvnw": f["kv_norm_w"].astype(np.float32).reshape(1, KVLR),
        "wqab": f["wq_a_b"].astype(np.float32).reshape(1, QLR),
        "wqbb": f["wq_b_b"].astype(np.float32).reshape(1, H * (DN + DR)),
        "wkvab": f["wkv_a_b"].astype(np.float32).reshape(1, KVLR + DR),
        "wob": f["wo_b"].astype(np.float32).reshape(1, DIM),
        "cosq": np.tile(cos, H).reshape(1, H * 32),
        "sinq": np.tile(sin, H).reshape(1, H * 32),
    }
    in_maps = []
    for b in range(B):
        m = dict(shared)
        m["xT16"] = np.ascontiguousarray(x[b].reshape(16, 128).T).astype(npbf)
        kvT4 = np.zeros((4, 128, T), dtype=npbf)
        kvT4[:, :, :TP] = np.ascontiguousarray(kvp[b].T).reshape(4, 128, TP)
        m["kvT4"] = kvT4
        peT = np.zeros((64, T), dtype=npbf)
        peT[:, :TP] = pep_[b].T
        m["peT"] = peT
        in_maps.append(m)
    return in_maps


def run(inputs, trace=False, tmpdir=None):
    nc = _build()
    in_maps = _prep_inputs(inputs)
    res = bass_utils.run_bass_kernel_spmd(
        nc, in_maps, core_ids=list(range(N_CORES)), trace=trace, tmpdir=tmpdir,
    )
    out = np.zeros((B, S, DIM), dtype=np.float32)
    for b in range(B):
        out[b, 0, :] = res.results[b]["out"][0]
    return out, res


def kernel(**inputs) -> np.ndarray:
    out, _ = run(inputs, trace=False)
    return out


# revision 18
# speedup vs baseline: 1.5909x; 1.5909x over previous
"""MLA decode kernel for 8 TRN2 NeuronCores (v5).

Sharding: batch-parallel - core b handles batch element b (B=8, n_cores=8).
All weights replicated per core, host-pretiled into partition-major
contiguous layouts so every DMA moves 4-18KB contiguous lines per
partition (the baseline's 1-2KB strided lines held HBM at ~33% MBU).

Key changes vs baseline:
- kv cache fed ONLY in [c,t] layout (kvT); the [t,c] layout for the value
  matmul is built on-chip with the DMA XBAR transpose (InstDmaTransposeAnt,
  14ns per 16x128 tile) - saves 8.4MB of HBM traffic per core.
- exp(scores) transposed for the value matmul via XBAR too, replacing
  64 PE transposes + 64 scalar copies.
- q_norm folded into wq_b rows on host; absorption restructured to
  16 weight-streaming matmuls + one XBAR instead of 64 stationary-load
  matmuls.
"""
import numpy as np
import ml_dtypes

import concourse.bacc as bacc
import concourse.mybir as mybir
from concourse import bass_utils
from concourse.tile import TileContext
from concourse.masks import make_identity

BF = mybir.dt.bfloat16
F32 = mybir.dt.float32
npbf = ml_dtypes.bfloat16

N_CORES = 8
B, S, DIM = 8, 1, 2048
H = 16
QLR, KVLR = 1536, 512
DN, DR, DV = 128, 64, 128
TP = 8191
T = TP + 1           # 8192 padded; col 8191 = injected new token
SCALE = float((DN + DR) ** -0.5)
EPS = 1e-6
TBW = 1024           # t-block width for kvT streaming
NTB = T // TBW       # 4 blocks
NSB = T // 512       # 16 score sub-blocks

_NC_CACHE = {}


def _build():
    if "nc" in _NC_CACHE:
        return _NC_CACHE["nc"]
    nc = bacc.Bacc("TRN2", target_bir_lowering=False, debug=False,
                   num_devices=N_CORES)
    I = {}

    def inp(name, shape, dt=BF):
        I[name] = nc.dram_tensor(name, shape, dt, kind="ExternalInput")
        return I[name]

    inp("xT16", [128, 16])
    inp("kvT4", [4, 128, T])
    inp("peT", [64, T])
    inp("wqa_p", [128, 3, 16, 512])
    inp("wqb_p", [128, 6, 12, 512])
    inp("wkva_p", [128, 2, 16, 288])
    inp("wbk_p", [128, 16, 512])
    inp("wbv_p", [128, 4, 2048])
    inp("woT_p", [128, 4, 16, 512])
    inp("cbank", [1, 8768], F32)
    out_d = nc.dram_tensor("out", [1, DIM], F32, kind="ExternalOutput")

    with TileContext(nc) as tc:
        _program(nc, tc, I, out_d)
    nc.compile()
    _NC_CACHE["nc"] = nc
    return nc


def _program(nc, tc, I, out_d):
    AL = mybir.AluOpType
    AF = mybir.ActivationFunctionType

    with (
        tc.tile_pool(name="consts", bufs=1) as cp,
        tc.tile_pool(name="wstream", bufs=2) as wp,
        tc.tile_pool(name="wconst", bufs=1) as wc,
        tc.tile_pool(name="kvTp", bufs=2) as kvTp,
        tc.tile_pool(name="kvnp", bufs=2) as kvnp,
        tc.tile_pool(name="attn", bufs=3) as atp,
        tc.tile_pool(name="ps_scores", bufs=2, space="PSUM") as pps,
        tc.tile_pool(name="ps_acc", bufs=1, space="PSUM") as ppa,
        tc.tile_pool(name="ps_tr", bufs=2, space="PSUM") as ppt,
        tc.tile_pool(name="ps_stage", bufs=1, space="PSUM") as ppg,
    ):
        id_f = cp.tile([128, 128], F32)
        make_identity(nc, id_f[:])
        id_bf = cp.tile([128, 128], BF)
        make_identity(nc, id_bf[:])

        def load_const(name, dt=F32):
            t = cp.tile(list(I[name].shape), dt, tag=name)
            nc.sync.dma_start(out=t[:], in_=I[name].ap())
            return t

        xT = load_const("xT16", BF)
        cb = load_const("cbank")
        kvnw = cb[:, 0:512]
        wqab = cb[:, 512:2048]
        wqbb = cb[:, 2048:5120]
        wkvab = cb[:, 5120:5696]
        wob = cb[:, 5696:7744]
        cosq = cb[:, 7744:8256]
        sinq = cb[:, 8256:8768]

        # big resident tiles
        peT_sb = wc.tile([64, T], BF)
        nc.sync.dma_start(out=peT_sb[:], in_=I["peT"].ap())
        wbk_sb = wc.tile([128, 16, 512], BF)
        nc.sync.dma_start(out=wbk_sb[:], in_=I["wbk_p"].ap())
        # ---- GEMV: y[1,M] f32 = x^T @ W (+bias); W host-pretiled ----
        def gemv_pret(w_name, nblocks, nk, mwtot, bias_sb, out_sb, x_sb):
            wd = I[w_name].ap()
            off = 0
            for mb in range(nblocks):
                wt = wp.tile([128, 16, 512], BF, tag="wstream")
                nc.sync.dma_start(out=wt[:, :nk, :mwtot], in_=wd[:, mb])
                for mb0 in range(0, mwtot, 512):
                    mw = min(512, mwtot - mb0)
                    ps = ppg.tile([1, 512], F32, tag="stage")
                    for kc in range(nk):
                        nc.tensor.matmul(
                            ps[:, :mw], x_sb[:, kc:kc + 1],
                            wt[:, kc, mb0:mb0 + mw],
                            start=(kc == 0), stop=(kc == nk - 1),
                        )
                    nc.vector.tensor_tensor(
                        out=out_sb[:, off + mb0:off + mb0 + mw], in0=ps[:, :mw],
                        in1=bias_sb[:, off + mb0:off + mb0 + mw], op=AL.add,
                    )
                off += mwtot

        def rmsnorm(in_view, N, w_sb, out_sb, tag):
            sq = cp.tile([1, 1536], F32, tag="scratch")
            ssq = cp.tile([1, 1], F32, tag=f"ssq{tag}")
            nc.scalar.activation(out=sq[:, :N], in_=in_view, func=AF.Square,
                                 accum_out=ssq[:])
            ms = cp.tile([1, 1], F32, tag=f"ms{tag}")
            nc.vector.tensor_scalar(out=ms[:], in0=ssq[:], scalar1=1.0 / N,
                                    scalar2=EPS, op0=AL.mult, op1=AL.add)
            sd = cp.tile([1, 1], F32, tag=f"sd{tag}")
            nc.scalar.activation(out=sd[:], in_=ms[:], func=AF.Sqrt)
            rstd = cp.tile([1, 1], F32, tag=f"rstd{tag}")
            nc.vector.reciprocal(out=rstd[:], in_=sd[:])
            if w_sb is not None:
                tmp = cp.tile([1, 1536], F32, tag="scratch")
                nc.vector.tensor_tensor(out=tmp[:, :N], in0=in_view,
                                        in1=w_sb[:, :N], op=AL.mult)
                src = tmp[:, :N]
            else:
                src = in_view
            nc.vector.tensor_scalar(out=out_sb, in0=src,
                                    scalar1=rstd[:], scalar2=None, op0=AL.mult)

        def trans_row(in_view, n, ps_out):
            nc.tensor.transpose(ps_out, in_view, id_f[0:1, 0:1])

        # ================= Q branch =================
        qa = cp.tile([1, QLR], F32)
        gemv_pret("wqa_p", 3, 16, 512, wqab, qa, xT)
        qan = cp.tile([1, QLR], F32)
        rmsnorm(qa[:], QLR, None, qan[:], "qa")

        pt_qa = ppt.tile([128, 64], F32, tag="tr")
        for kc in range(12):
            trans_row(qan[:, kc * 128:(kc + 1) * 128], 128,
                      pt_qa[:, kc:kc + 1])
        qaT = cp.tile([128, 12], BF)
        nc.scalar.copy(out=qaT[:], in_=pt_qa[:, :12])

        q = cp.tile([1, H * (DN + DR)], F32)
        gemv_pret("wqb_p", 6, 12, 512, wqbb, q, qaT)

        # rope(q_pe) across all heads
        qv = q[:].rearrange("b (h r) -> b h r", h=H)
        xr = qv[:, :, 128:192:2]
        xi = qv[:, :, 129:192:2]
        cosv = cosq.rearrange("b (h j) -> b h j", h=H)
        sinv = sinq.rearrange("b (h j) -> b h j", h=H)
        rp = cp.tile([1, H * DR], F32)
        rpv = rp[:].rearrange("b (h r) -> b h r", h=H)
        s1 = cp.tile([1, 512], F32, tag="rs1")
        s2 = cp.tile([1, 512], F32, tag="rs2")
        s1v = s1[:].rearrange("b (h j) -> b h j", h=H)
        s2v = s2[:].rearrange("b (h j) -> b h j", h=H)
        nc.vector.tensor_tensor(out=s1v, in0=xi, in1=sinv, op=AL.mult)
        nc.vector.tensor_tensor(out=s2v, in0=xr, in1=cosv, op=AL.mult)
        nc.vector.tensor_tensor(out=rpv[:, :, 0:64:2], in0=s2v, in1=s1v,
                                op=AL.subtract)
        nc.vector.tensor_tensor(out=s1v, in0=xr, in1=sinv, op=AL.mult)
        nc.vector.tensor_tensor(out=s2v, in0=xi, in1=cosv, op=AL.mult)
        nc.vector.tensor_tensor(out=rpv[:, :, 1:64:2], in0=s1v, in1=s2v,
                                op=AL.add)

        # q_nope columns -> qnT [128, 16] bf16
        pt_qn = ppt.tile([128, 64], F32, tag="tr")
        for h in range(H):
            trans_row(q[:, h * 192:h * 192 + 128], 128, pt_qn[:, h:h + 1])
        qnT = cp.tile([128, H], BF)
        nc.scalar.copy(out=qnT[:], in_=pt_qn[:, :H])

        # absorption: qT[cc][128c, 16h] = sum_d wbk[h,d,c] q_nope[h,d]
        qTx = cp.tile([128, 4, 16], BF)
        for cc in range(4):
            ps_ab = ppt.tile([128, 64], F32, tag="tr")
            for h in range(H):
                nc.tensor.matmul(
                    ps_ab[:, h:h + 1],
                    wbk_sb[:, h, cc * 128:(cc + 1) * 128],
                    qnT[:, h:h + 1], start=True, stop=True,
                )
            nc.scalar.copy(out=qTx[:, cc, :], in_=ps_ab[:, :H])

        # q_pe transposed -> qpT [64, 16]
        pt_qp = ppt.tile([128, 64], F32, tag="tr")
        for h in range(H):
            trans_row(rp[:, h * 64:(h + 1) * 64], 64, pt_qp[:64, h:h + 1])
        qpT = cp.tile([64, H], BF)
        nc.scalar.copy(out=qpT[:], in_=pt_qp[:64, :H])

        # ================= KV branch (new token) =================
        kvpe = cp.tile([1, KVLR + DR], F32)
        gemv_pret("wkva_p", 2, 16, 288, wkvab, kvpe, xT)
        kvn_f = cp.tile([1, KVLR], F32)
        rmsnorm(kvpe[:, :KVLR], KVLR, kvnw, kvn_f[:], "kv")

        kpe = cp.tile([1, DR], F32)
        kxr = kvpe[:, KVLR + 0:KVLR + 64:2]
        kxi = kvpe[:, KVLR + 1:KVLR + 64:2]
        ks1 = cp.tile([1, 32], F32, tag="krs1")
        ks2 = cp.tile([1, 32], F32, tag="krs2")
        nc.vector.tensor_tensor(out=ks1[:], in0=kxi, in1=sinq[:, :32], op=AL.mult)
        nc.vector.tensor_tensor(out=ks2[:], in0=kxr, in1=cosq[:, :32], op=AL.mult)
        nc.vector.tensor_tensor(out=kpe[:, 0:64:2], in0=ks2[:], in1=ks1[:],
                                op=AL.subtract)
        nc.vector.tensor_tensor(out=ks1[:], in0=kxr, in1=sinq[:, :32], op=AL.mult)
        nc.vector.tensor_tensor(out=ks2[:], in0=kxi, in1=cosq[:, :32], op=AL.mult)
        nc.vector.tensor_tensor(out=kpe[:, 1:64:2], in0=ks1[:], in1=ks2[:],
                                op=AL.add)

        kvn_bf = cp.tile([1, KVLR], BF)
        nc.scalar.copy(out=kvn_bf[:], in_=kvn_f[:])
        pt_kv = ppt.tile([128, 64], F32, tag="tr")
        for cc in range(4):
            trans_row(kvn_f[:, cc * 128:(cc + 1) * 128], 128,
                      pt_kv[:, cc:cc + 1])
        trans_row(kpe[:], 64, pt_kv[:64, 4:5])
        kvnT = cp.tile([128, 4], BF)
        nc.scalar.copy(out=kvnT[:], in_=pt_kv[:, :4])
        kpeT = cp.tile([64, 1], BF)
        nc.scalar.copy(out=kpeT[:], in_=pt_kv[:64, 4:5])

        # inject new token into pe cache (t=8191)
        nc.vector.tensor_copy(out=peT_sb[:, T - 1:T], in_=kpeT[:])

        # ================= attention =================
        den = cp.tile([H, NSB], F32)
        po = ppa.tile([H, 512], F32)
        kvT_d = I["kvT4"].ap()
        n_mm2 = 64
        mm2_i = 0
        for tb in range(NTB):
            t0 = tb * TBW
            kvTt = kvTp.tile([128, 4, TBW], BF, tag="kvT")
            nc.sync.dma_start(
                out=kvTt[:],
                in_=kvT_d[:, :, t0:t0 + TBW].rearrange("c p t -> p c t"),
            )
            # build kvn for this block: ONE XBAR [128,4096] -> [128,(cc,tc),128]
            kvn_sb = kvnp.tile([128, 4, TBW // 128, 128], BF, tag="kvn")
            xq = nc.scalar if tb % 2 == 0 else nc.sync
            xq.dma_start(
                out=kvn_sb[:],
                in_=kvTt[:].rearrange("p c t -> p (c t)"), transpose=True,
            )
            if tb == NTB - 1:
                # inject new token column (t=8191) into kvT and kvn
                for cc in range(4):
                    nc.vector.tensor_copy(out=kvTt[:, cc, TBW - 1:TBW],
                                          in_=kvnT[:, cc:cc + 1])
                nc.sync.dma_start(
                    out=kvn_sb[127:128, :, TBW // 128 - 1, :],
                    in_=kvn_bf[:].rearrange("a (c m) -> a c m", c=4),
                )

            for s in range(TBW // 512):
                sb_i = tb * (TBW // 512) + s
                ps = pps.tile([H, 512], F32, tag="scores")
                for cc in range(4):
                    nc.tensor.matmul(
                        ps[:], qTx[:, cc, :], kvTt[:, cc, s * 512:(s + 1) * 512],
                        start=(cc == 0), stop=False,
                    )
                nc.tensor.matmul(ps[:], qpT[:],
                                 peT_sb[:, t0 + s * 512:t0 + (s + 1) * 512],
                                 start=False, stop=True)
                ex = atp.tile([H, 512], BF, tag="exp")
                nc.scalar.activation(out=ex[:], in_=ps[:], func=AF.Exp,
                                     scale=SCALE,
                                     accum_out=den[:, sb_i:sb_i + 1])
                ptr = ppt.tile([128, 64], BF, tag="trb")
                for u in range(4):
                    nc.tensor.transpose(ptr[:, u * 16:(u + 1) * 16],
                                        ex[:, u * 128:(u + 1) * 128],
                                        id_bf[0:H, 0:H])
                exT = atp.tile([128, 64], BF, tag="expT")
                nc.scalar.copy(out=exT[:], in_=ptr[:])
                for u in range(4):
                    nc.tensor.matmul(
                        po[:], exT[:, u * 16:(u + 1) * 16],
                        kvn_sb[:, :, s * 4 + u, :],
                        start=(mm2_i == 0), stop=(mm2_i == n_mm2 - 1),
                        skip_group_check=True,
                    )
                    mm2_i += 1

        den1 = cp.tile([H, 1], F32)
        nc.vector.tensor_reduce(out=den1[:], in_=den[:],
                                axis=mybir.AxisListType.X, op=AL.add)
        dinv = cp.tile([H, 1], F32)
        nc.vector.reciprocal(out=dinv[:], in_=den1[:])
        oln = cp.tile([H, 512], BF)
        nc.vector.tensor_scalar(out=oln[:], in0=po[:], scalar1=dinv[:],
                                scalar2=None, op0=AL.mult)

        # olT via XBAR: [16, 512] -> [128, 4, 16]
        olT = cp.tile([128, 4, 16], BF)
        nc.scalar.dma_start(out=olT[:], in_=oln[:], transpose=True)

        # V projection -> oT [128d, 16h] bf16
        wbv_sb = wc.tile([128, 4, 2048], BF)
        nc.sync.dma_start(out=wbv_sb[:], in_=I["wbv_p"].ap())
        ps_vo = ppt.tile([128, 64], F32, tag="tr")
        for h in range(H):
            for cc in range(4):
                nc.tensor.matmul(
                    ps_vo[:, h:h + 1],
                    wbv_sb[:, cc, h * 128:(h + 1) * 128],
                    olT[:, cc, h:h + 1],
                    start=(cc == 0), stop=(cc == 3),
                )
        oT = cp.tile([128, H], BF)
        nc.scalar.copy(out=oT[:], in_=ps_vo[:, :H])

        # wo projection -> out [1, 2048] f32
        woT_d = I["woT_p"].ap()
        for mb in range(4):
            wt = wp.tile([128, 16, 512], BF, tag="wstream")
            nc.sync.dma_start(out=wt[:], in_=woT_d[:, mb])
            ps = ppg.tile([1, 512], F32, tag="stage")
            for h in range(H):
                nc.tensor.matmul(ps[:], oT[:, h:h + 1], wt[:, h, :],
                                 start=(h == 0), stop=(h == H - 1))
            ob = cp.tile([1, 512], F32, tag="outb")
            nc.vector.tensor_tensor(
                out=ob[:], in0=ps[:],
                in1=wob[:, mb * 512:(mb + 1) * 512], op=AL.add,
            )
            nc.sync.dma_start(out=out_d.ap()[:, mb * 512:(mb + 1) * 512],
                              in_=ob[:])


def _prep_inputs(inputs):
    f = {k: np.asarray(v) for k, v in inputs.items()}
    x = f["x"].astype(np.float32).reshape(B, DIM)
    kvp = f["kv_cache_prefix"].astype(np.float32)
    pep_ = f["pe_cache_prefix"].astype(np.float32)
    cos = f["freqs_cos"].astype(np.float32).reshape(-1)[:32]
    sin = f["freqs_sin"].astype(np.float32).reshape(-1)[:32]

    wq_a = f["wq_a_w"].astype(np.float32)          # [1536, 2048]
    wq_b = f["wq_b_w"].astype(np.float32)          # [3072, 1536]
    qnw = f["q_norm_w"].astype(np.float32)
    wkv_a = f["wkv_a_w"].astype(np.float32)        # [576, 2048]
    wkv_b = f["wkv_b_w"].astype(np.float32).reshape(H, DN + DV, KVLR)
    wo = f["wo_w"].astype(np.float32)              # [2048, 2048]

    # pretiled layouts: [p, mb, k, m] with per-(p) contiguous (mb,k,m) lines
    wqa_p = np.ascontiguousarray(
        wq_a.reshape(3, 512, 16, 128).transpose(3, 0, 2, 1)).astype(npbf)
    wqbw = wq_b * qnw[None, :]                     # fold q_norm into wq_b
    wqb_p = np.ascontiguousarray(
        wqbw.reshape(6, 512, 12, 128).transpose(3, 0, 2, 1)).astype(npbf)
    wkva_p = np.ascontiguousarray(
        wkv_a.reshape(2, 288, 16, 128).transpose(3, 0, 2, 1)).astype(npbf)
    wbk_p = np.ascontiguousarray(
        wkv_b[:, :DN].transpose(1, 0, 2)).astype(npbf)   # [128d, 16h, 512c]
    wbv_p = np.ascontiguousarray(
        wkv_b[:, DN:].reshape(H, DV, 4, 128).transpose(3, 2, 0, 1)
        .reshape(128, 4, H * DV)).astype(npbf)           # [128c, 4cc, (h,d)]
    woT_p = np.ascontiguousarray(
        wo.T.reshape(16, 128, 4, 512).transpose(1, 2, 0, 3)).astype(npbf)

    shared = {
        "wqa_p": wqa_p, "wqb_p": wqb_p, "wkva_p": wkva_p,
        "wbk_p": wbk_p, "wbv_p": wbv_p, "woT_p": woT_p,
        "cbank": np.concatenate([
            f["kv_norm_w"].astype(np.float32).reshape(1, KVLR),
            f["wq_a_b"].astype(np.float32).reshape(1, QLR),
            f["wq_b_b"].astype(np.float32).reshape(1, H * (DN + DR)),
            f["wkv_a_b"].astype(np.float32).reshape(1, KVLR + DR),
            f["wo_b"].astype(np.float32).reshape(1, DIM),
            np.tile(cos, H).reshape(1, H * 32).astype(np.float32),
            np.tile(sin, H).reshape(1, H * 32).astype(np.float32),
        ], axis=1),
    }
    in_maps = []
    for b in range(B):
        m = dict(shared)
        m["xT16"] = np.ascontiguousarray(x[b].reshape(16, 128).T).astype(npbf)
        kvT4 = np.zeros((4, 128, T), dtype=npbf)
        kvT4[:, :, :TP] = np.ascontiguousarray(kvp[b].T).reshape(4, 128, TP)
        m["kvT4"] = kvT4
        peT = np.zeros((64, T), dtype=npbf)
        peT[:, :TP] = pep_[b].T
        m["peT"] = peT
        in_maps.append(m)
    return in_maps


def run(inputs, trace=False, tmpdir=None):
    nc = _build()
    in_maps = _prep_inputs(inputs)
    res = bass_utils.run_bass_kernel_spmd(
        nc, in_maps, core_ids=list(range(N_CORES)), trace=trace, tmpdir=tmpdir,
    )
    out = np.zeros((B, S, DIM), dtype=np.float32)
    for b in range(B):
        out[b, 0, :] = res.results[b]["out"][0]
    return out, res


def kernel(**inputs) -> np.ndarray:
    out, _ = run(inputs, trace=False)
    return out
